# revision 26
# baseline (speedup 1.0000x reference)
"""Trainium2 Bass kernel: 4096x4096 single-channel 3x3 VALID conv + bias.

Sharding: 2x4 spatial grid over 8 cores. Core c = (rb, cb) = (c//4, c%4)
computes output rows [2047*rb, +2047) x cols [1024*cb, +1024) (cb=3: 1022
valid). Input shard: [2049, 1026] bf16 (halo included, zero-padded right
edge for cb=3). All I/O is bf16 (5e-3 rel err vs the 2e-2 budget); host
converts fp32<->bf16.

Per core: 17 stripes of <=126 output rows, all x/y tiles SBUF-resident.
Per stripe, per 512-col PSUM bank (pairs rotate mod 4), 3 bf16 matmuls
(kernel column dj, rhs shifted by dj) against 128x126 band matrices
accumulate all 9 taps; the PE stream (102 x 512 columns @ ~2.4GHz,
LDWEIGHTS overlapped) is the compute floor. ScalarE/VectorE evacuate the
two banks fusing +bias and the bf16 cast; stores alternate rings.

Start path: stripe 0+1's input and the bias ride INSIDE the band-matrix
tensor, so one two-part load on the otherwise-idle sync ring gates the
first two stripes with single completions; the PE runs data-independent
warmup matmuls on a zeroed scratch tile until it lands (an idle PE
resets the DVFS ramp, ~2-4us of half-speed matmuls).

Timing model from NTFF traces (v1..v3 experiments):
- exec_time = first-user-instruction -> last-postamble-instruction. The
  NRT postamble is ~60 ops/engine (~7us) and its size is FIXED: it does
  not scale with declared dma rings/queues (v2) or semaphore count (v3).
  Do not touch nc.m.queues: reshuffling ring IDs disables the SWDGE
  ring's adaptive queue balancing (engines 64-71 double-load, +12us).
- Per-dma_start issue is ~0.6-0.8us of ring-engine time and a ring
  serializes descriptor-gen, so a chain of separate startup loads feeds
  the PE ~2.7us later than the single combined load (v3 lesson).
- The ring hands chunks (~64KB) to DMA engines; a chunk moves at
  ~23GB/s, so a stripe load's completion wall is ~2.8us.
- then_inc(sem,16) posts ~15 increments at descriptor-gen time, the
  16th at true completion; waits gating CORRECTNESS use exact multiples
  of 16 totals whose contributors are >=4 stripes of ring work apart.
- Dropping the final st_sem wait does NOT shorten the tail (the NRT
  postamble's quiesce serializes either way), so it is kept for safety.
- ScalarE evacuates bank A, VectorE bank B — never split one bank's
  evac across two engines: concurrent same-bank PSUM reads hang the
  device (as does any read crossing a 512-float bank boundary).
- The PE clock is a per-run lottery: most runs settle at ~2.37GHz
  (216ns per 512-col stream), an occasional run caps at ~1.97GHz
  (260ns) and everything — engines, postamble — stretches ~20%.
"""

import numpy as np

import concourse.bass as bass
import concourse.mybir as mybir
from concourse.bass_utils import run_bass_kernel_spmd

H = W = 4096
KH = KW = 3
OH = OW = H - KH + 1   # 4094
NCORES = 8
GRID_R, GRID_C = 2, 4
BROWS = 2047           # output rows per core block
BCOLS = 1024           # output cols per core block (cb=3: 1022 valid)
IN_ROWS = BROWS + KH - 1   # 2049
IN_COLS = BCOLS + KW - 1   # 1026
SPLIT = 512            # scalar evacuates [0:512), vector [512:1024) (bank-aligned)

HEIGHTS = [126] * 16 + [31]
assert sum(HEIGHTS) == BROWS
STRIPES = []           # (out_row_start, out_rows, in_rows)
_r = 0
for _h in HEIGHTS:
    STRIPES.append((_r, _h, _h + KH - 1))
    _r += _h
N_S = len(STRIPES)     # 17

# mb layout: band 0:378, bias col 378, stripe-0 input 379:1405, stripe-1
# input 1405:2431. Loaded as two dmas split at MBX_X1 (band+bias+x0 ->
# ldA, x1 -> ins[1]) so one ring pass gates the first two stripes.
BIAS_COL = KW * 126          # 378
MBX_X0 = BIAS_COL + 1        # 379
MBX_X1 = MBX_X0 + IN_COLS    # 1405
MBX_COLS = MBX_X1 + IN_COLS  # 2431
Q_SYNC2 = [(2, 0, 64), (2, 64, 128)]

# st_sem: 16 per store (16 full + 2 last pieces) + 16 for the pool-ring
# priming dummy
ST_FINAL = 16 * (N_S - 1 + 2 + 1)

_cached = None


def _in_sem_thresholds():
    """Cumulative then_inc(16) totals per rotating in-sem, per stripe."""
    cum = [0, 0, 0, 0]
    thresh = [0] * N_S
    pieces = {s: 0 for s in range(N_S)}
    pieces[1] = 1
    for s, _, _ in Q_SYNC2:
        pieces[s] += 1
    for s in range(3, N_S):
        pieces[s] += 1
    for s in range(N_S):
        cum[s % 4] += 16 * pieces[s]
        thresh[s] = cum[s % 4]
    return thresh


def _build():
    nc = bass.Bass()
    x_d = nc.dram_tensor("x", [IN_ROWS, IN_COLS], mybir.dt.bfloat16, kind="ExternalInput")
    mb_d = nc.dram_tensor("mb", [128, MBX_COLS], mybir.dt.bfloat16, kind="ExternalInput")
    y_d = nc.dram_tensor("y", [BROWS, BCOLS], mybir.dt.bfloat16, kind="ExternalOutput")

    thresh = _in_sem_thresholds()

    import contextlib
    with contextlib.ExitStack() as st:
        ec = st.enter_context
        xb = [None, None] + [
            ec(nc.sbuf_tensor(f"x{s}", [128, IN_COLS], mybir.dt.bfloat16))
            for s in range(2, N_S)]
        yb = [ec(nc.sbuf_tensor(f"y{s}", [128, BCOLS], mybir.dt.bfloat16))
              for s in range(N_S)]
        mb = ec(nc.sbuf_tensor("mb_sb", [128, MBX_COLS], mybir.dt.bfloat16))
        warm = ec(nc.sbuf_tensor("warm", [128, 512], mybir.dt.bfloat16))
        bvf = ec(nc.sbuf_tensor("bvf", [128, 1], mybir.dt.float32))
        scr = ec(nc.sbuf_tensor("scr", [1, 8], mybir.dt.bfloat16))
        ps = ec(nc.psum_tensor([128, 4096], mybir.dt.float32))
        wm = ec(nc.semaphore("wm"))
        bvr = ec(nc.semaphore("bvr"))
        ldA = ec(nc.semaphore("ldA"))
        ins = [ec(nc.semaphore(f"in{q}")) for q in range(4)]
        pe_sem = ec(nc.semaphore("pe_sem"))
        evA = ec(nc.semaphore("evA"))
        evB = ec(nc.semaphore("evB"))
        st_sem = ec(nc.semaphore("st_sem"))
        # skip GpSimd's ~2.6us block-exit dge_drain: it gates the barrier
        # later than the st_sem store-completion wait does; the sem-only
        # barrier + NRT teardown still quiesce everything
        blk = ec(nc.Block(no_gpsimd_drain=True))

        def load_piece(eng, s, lo, hi):
            r0 = STRIPES[s][0]
            eng.dma_start(
                xb[s][lo:hi, :], x_d.ap()[r0 + lo:r0 + hi, :]
            ).then_inc(ins[s % 4], 16)

        bias_ap = lambda orows: mb[0:orows, BIAS_COL:BIAS_COL + 1]

        @blk.gpsimd
        def _(gpsimd):
            # priming dummy shifts this ring's engine pointer off engine 0
            # (whose queues the sync ring's combined load claims first)
            gpsimd.dma_start(
                scr[0:1, 6:7], mb_d.ap()[0:1, 0:1]
            ).then_inc(st_sem, 16)
            # let the combined mb+x0+x1 load claim engines first: its ring
            # chunks race this ring's bulk for the shared DMA engines
            gpsimd.wait_ge(ldA, 8)
            for s in range(4, N_S):
                load_piece(gpsimd, s, 0, STRIPES[s][2])
            for s, (r0, orows, irows) in enumerate(STRIPES):
                if s % 2 != 0:
                    continue
                if s == N_S - 1:
                    # pre-wake: a parked engine takes ~1.3us to resume after
                    # its sem fires; waking on the last stripe's first bank
                    # absorbs the latency before the evac gates are checked.
                    # (No dummy dma here: its ~0.6us issue used to DELAY the
                    # final store past the evB fire.)
                    gpsimd.wait_ge(pe_sem, 2 * s + 1)
                    # row-split across rings: full-width 2KB lines move at
                    # ~80ns/desc vs ~155ns for half-width 1KB lines
                    gpsimd.wait_ge(evA, s + 1)
                    gpsimd.wait_ge(evB, s + 1)
                    gpsimd.dma_start(
                        y_d.ap()[r0 + 16:r0 + orows, :], yb[s][16:orows, 0:BCOLS]
                    ).then_inc(st_sem, 16)
                else:
                    gpsimd.wait_ge(evA, s + 1)
                    gpsimd.wait_ge(evB, s + 1)
                    gpsimd.dma_start(
                        y_d.ap()[r0:r0 + orows, :], yb[s][0:orows, 0:BCOLS]
                    ).then_inc(st_sem, 16)

        @blk.scalar
        def _(scalar):
            scalar.wait_ge(ldA, 16)
            # trigger the lazy ACT table load AND widen the bf16 bias
            # column to fp32 for VectorE (tensor_scalar requires fp32)
            nc.scalar.activation(
                out=bvf[0:128, 0:1], in_=mb[0:128, BIAS_COL:BIAS_COL + 1],
                func=mybir.ActivationFunctionType.Identity,
                bias=0.0, scale=1.0,
            ).then_inc(bvr, 1)
            for s, (r0, orows, irows) in enumerate(STRIPES):
                p = s % 4
                # bank A (cols 0:512 of the pair) is complete at 2s+1;
                # waking here overlaps the park-wakeup with bank B's MMs
                scalar.wait_ge(pe_sem, 2 * s + 1)
                nc.scalar.activation(
                    out=yb[s][0:orows, 0:SPLIT],
                    in_=ps[0:orows, 1024 * p:1024 * p + SPLIT],
                    func=mybir.ActivationFunctionType.Identity,
                    bias=bias_ap(orows),
                    scale=1.0,
                ).then_inc(evA, 1)
            # last stripe's bank B too: scalar is already running (no
            # park-wakeup), vector would wake ~1.3us after the final MM
            s, (r0, orows, irows) = N_S - 1, STRIPES[N_S - 1]
            p = s % 4
            scalar.wait_ge(pe_sem, 2 * s + 2)
            nc.scalar.activation(
                out=yb[s][0:orows, SPLIT:BCOLS],
                in_=ps[0:orows, 1024 * p + SPLIT:1024 * p + BCOLS],
                func=mybir.ActivationFunctionType.Identity,
                bias=bias_ap(orows),
                scale=1.0,
            ).then_inc(evB, 1)

        @blk.vector
        def _(vector):
            # zero scratch for the PE's data-independent p-state warmup
            nc.vector.memset(warm[0:128, 0:512], 0).then_inc(wm, 1)
            vector.wait_ge(bvr, 1)
            for s, (r0, orows, irows) in enumerate(STRIPES):
                if s == N_S - 1:
                    continue
                p = s % 4
                vector.wait_ge(pe_sem, 2 * s + 2)
                nc.vector.tensor_scalar_add(
                    out=yb[s][0:orows, SPLIT:BCOLS],
                    in0=ps[0:orows, 1024 * p + SPLIT:1024 * p + BCOLS],
                    scalar1=bvf[0:orows, 0:1],
                ).then_inc(evB, 1)

        @blk.tensor
        def _(tensor):
            tensor.wait_ge(wm, 1)
            # p-state warmup on zeroed scratch while the combined load
            # lands; the last rounds are paced by its progress increments
            # so the PE never idles (an idle gap resets the clock ramp)
            def warmup(n):
                for _ in range(n):
                    nc.tensor.matmul(
                        ps[0:126, 2048:2560],
                        warm[0:128, 0:126],
                        warm[0:128, 0:512],
                        start=True, stop=True,
                    )
            warmup(7)
            tensor.wait_ge(ldA, 16)
            for s, (r0, orows, irows) in enumerate(STRIPES):
                p = s % 4
                if thresh[s]:
                    tensor.wait_ge(ins[p], thresh[s])
                if s >= 4:
                    tensor.wait_ge(evA, s - 3)
                    tensor.wait_ge(evB, s - 3)
                if s == 0:
                    rhs, rc = mb, MBX_X0
                elif s == 1:
                    rhs, rc = mb, MBX_X1
                else:
                    rhs, rc = xb[s], 0
                for h in range(2):
                    c0 = 1024 * p + 512 * h
                    mm = None
                    for dj in range(KW):
                        mm = nc.tensor.matmul(
                            ps[0:orows, c0:c0 + 512],
                            mb[0:irows, dj * 126:dj * 126 + orows],
                            rhs[0:irows, rc + 512 * h + dj:rc + 512 * h + dj + 512],
                            start=(dj == 0),
                            stop=(dj == KW - 1),
                        )
                    mm.then_inc(pe_sem, 1)

        @blk.sync
        def _(sync):
            # dummy load shifts this ring's engine pointer off engine 0
            sync.dma_start(
                mb[0:128, 0:MBX_X1], mb_d.ap()[0:128, 0:MBX_X1]
            ).then_inc(ldA, 16)
            sync.dma_start(
                mb[0:128, MBX_X1:MBX_COLS], mb_d.ap()[0:128, MBX_X1:MBX_COLS]
            ).then_inc(ins[1], 16)
            for s, lo, hi in Q_SYNC2:
                load_piece(sync, s, lo, hi)
            # x3 rides the sync ring too: the SWDGE pool ring's ~3us
            # startup latency leaves stripe 3 short otherwise, and sync
            # has idle issue capacity until the first odd store (~T0+2.6)
            load_piece(sync, 3, 0, STRIPES[3][2])
            for s, (r0, orows, irows) in enumerate(STRIPES):
                if s % 2 == 0 and s != N_S - 1:
                    continue
                if s == N_S - 1:
                    sync.wait_ge(pe_sem, 2 * s + 1)
                    sync.wait_ge(evA, s + 1)
                    sync.wait_ge(evB, s + 1)
                    sync.dma_start(
                        y_d.ap()[r0:r0 + 16, :], yb[s][0:16, 0:BCOLS]
                    ).then_inc(st_sem, 16)
                else:
                    sync.wait_ge(evA, s + 1)
                    sync.wait_ge(evB, s + 1)
                    sync.dma_start(
                        y_d.ap()[r0:r0 + orows, :], yb[s][0:orows, 0:BCOLS]
                    ).then_inc(st_sem, 16)
            # hold the NEFF open until all stores land (dropping this wait
            # saved nothing: the post-stream tail stayed ~12us either way)
            sync.wait_ge(st_sem, ST_FINAL)

    return nc


def _host_prep(input, weight, bias):
    import ml_dtypes
    bf16 = ml_dtypes.bfloat16
    input = np.ascontiguousarray(np.asarray(input, dtype=np.float32).astype(bf16))
    weight = np.asarray(weight, dtype=np.float32)
    bias = np.asarray(bias, dtype=np.float32)

    # band matrices packed side by side: mb[:, dj*126+m] column m of M_dj,
    # M_dj[k, m] = weight[k-m, dj] for 0 <= k-m < KH; bias in col 378;
    # stripe 0+1 input appended per core so one two-part load gates the
    # first two stripes.
    band = np.zeros((128, KW * 126 + 1), dtype=np.float32)
    idx = np.arange(126)
    for dj in range(KW):
        for di in range(KH):
            band[idx + di, dj * 126 + idx] = weight[di, dj]
    band[:, BIAS_COL] = bias[0]
    band = band.astype(bf16)

    in_maps = []
    for c in range(NCORES):
        rb, cb = c // GRID_C, c % GRID_C
        r0, c0 = rb * BROWS, cb * BCOLS
        sl = input[r0:r0 + IN_ROWS, c0:c0 + IN_COLS]
        if sl.shape[1] < IN_COLS:
            sl = np.concatenate(
                [sl, np.zeros((sl.shape[0], IN_COLS - sl.shape[1]), bf16)], axis=1
            )
        sl = np.ascontiguousarray(sl)
        mbx = np.zeros((128, MBX_COLS), dtype=bf16)
        mbx[:, :MBX_X0] = band
        mbx[0:STRIPES[0][2], MBX_X0:MBX_X0 + IN_COLS] = sl[0:STRIPES[0][2]]
        r1 = STRIPES[1][0]
        mbx[0:STRIPES[1][2], MBX_X1:MBX_X1 + IN_COLS] = sl[r1:r1 + STRIPES[1][2]]
        in_maps.append({"x": sl, "mb": mbx})
    return in_maps


def _run(input, weight, bias, **spmd_kwargs):
    global _cached
    if _cached is None:
        _cached = _build()
    in_maps = _host_prep(input, weight, bias)
    res = run_bass_kernel_spmd(
        _cached, in_maps, core_ids=list(range(NCORES)), **spmd_kwargs
    )
    out = np.empty((OH, OW), dtype=np.float32)
    for c in range(NCORES):
        rb, cb = c // GRID_C, c % GRID_C
        r0, c0 = rb * BROWS, cb * BCOLS
        cols = min(BCOLS, OW - c0)
        out[r0:r0 + BROWS, c0:c0 + cols] = (
            res.results[c]["y"][:, :cols].astype(np.float32)
        )
    return out, res


def kernel(input, weight, bias):
    out, _ = _run(input, weight, bias)
    return out
